# revision 58
# baseline (speedup 1.0000x reference)
"""Trainium2 Bass kernel for nn_RelativeMultiHeadAttn (TransformerXL-style
relative multi-head attention).

Sharding: data-parallel over batch — core b handles batch element b (B=8).

Per-core math (S=512, D=1024, H=16 heads, HD=64):
  q = x @ Wq ; v = x @ Wv ; k_h = x[:, h*64:(h+1)*64]
  S_h   = (q_h + r_r_bias_h) @ k_h^T + shift((q_h + r_w_bias_h) @ pos^T)
  out_h = softmax(S_h) @ v_h

v3 design: scores are computed TRANSPOSED (S^T, k on partitions) so the
AV contraction needs no separate P-transpose stage:
  S^T = AC^T + BD^T
  AC^T[k,q] : matmul with lhsT = x^T slice, rhs = rwq  (same cost as AC)
  BD^T      : PE identity-transposes of the skew-read BD tile, accumulated
              straight onto AC^T in PSUM (replaces both the old P^T
              transposes AND the DVE BD-adds / P^T copies)
  P^T = exp(S^T)  (one ACT op per 2-bank PSUM chunk), directly consumable
        by the AV matmuls. Softmax denominators come from a ones-column
        appended to v (AV matmul N=65), so no accum_out needed.

The relative shift is the baseline's DRAM round-trip: per head a
[128, 4, 640] fp16 band of X = rwq2 @ pos^T is written contiguously and BD
is read back with a skewed access pattern (row stride 639, column offset
128) that lands each row's shifted 512-wide window densely.

All matmul operands are fp16/bf16 (10/8-bit mantissa ~ f32r accuracy for
fp16) which enables FWL fast weight loads — the baseline spent 139us in
f32r LDWEIGHTS — and halves DMA traffic. P is bf16 (exp can exceed fp16
range). Output is stored fp16 and cast to f32 on host.
"""

import numpy as np
import ml_dtypes

import concourse.bass as bass
import concourse.mybir as mybir
import concourse.tile as tile
from concourse.bass_utils import run_bass_kernel_spmd
from concourse.vector_clock import ScopedClock

B, S, D, H = 8, 512, 1024, 16
HD = D // H          # 64
QT = S // 128        # 4 q tiles
KT = D // 128        # 8 model-dim tiles
BAND = 640           # X band width per q-tile
POSW = 1024          # pos table width (2S)
CSKEW = 128          # uniform skew-read column offset
f32 = mybir.dt.float32
bf16 = mybir.dt.bfloat16
fp16 = mybir.dt.float16


# ---------------------------------------------------------------------------
# TileContext exit-drain workaround: this snapshot attaches every outstanding
# sem wait to one SP Drain, which walrus rejects ("Too many sync wait
# commands"). Split the waits across standalone SP nops instead.
def _drain_and_barrier_split(self, tick_clock, wait_clock):
    nc = self.nc
    probe = nc.sync.nop()
    wait_clock.add_sem_waits(probe.ins, ScopedClock({None: tick_clock.global_clock}))
    si = probe.ins.sync_info
    waits = list(si.on_wait) if si is not None else []
    if si is not None and len(waits) > 1:
        si.on_wait = [waits[0]]
        for w in waits[1:]:
            extra = nc.sync.nop()
            esi = extra.ins.sync_info
            if esi is None:
                extra.ins.sync_info = mybir.SyncInfo(on_wait=[w], on_update=[])
            else:
                esi.on_wait = [w]
    nc.sync.drain()
    nc.all_engine_barrier()
    assert self.sems is not None
    popped = nc._tile_sem_poison_stack.pop()
    assert popped is self._sem_poison
    nc.clear_and_free_semaphores(list(self.sems.allocated().values()))
    nc.all_engine_barrier()


tile.TileContext._drain_and_barrier = _drain_and_barrier_split

_wsplit_counter = [0]


def _split_excess_waits(nc, max_waits=1):
    """Walrus in this container rejects instructions carrying more than one
    sem wait ("Too many sync wait commands"), but Tile's wait-assignment pass
    can attach several. Move excess waits onto fresh NoOps inserted right
    before the instruction on the same engine."""
    for f in nc.m.functions:
        for bb in f.blocks:
            new_insts = []
            changed = False
            for inst in bb.instructions:
                si = inst.sync_info
                waits = list(si.on_wait) if si is not None else []
                if len(waits) > max_waits and inst.engine != mybir.EngineType.Unassigned:
                    for w in waits[:-max_waits]:
                        _wsplit_counter[0] += 1
                        nop = mybir.InstNoOp(
                            name=f"WSPLIT-{_wsplit_counter[0]}", ins=[], outs=[]
                        )
                        nop.engine = inst.engine
                        nop.sync_info = mybir.SyncInfo(on_wait=[w], on_update=[])
                        new_insts.append(nop)
                    si.on_wait = waits[-max_waits:]
                    changed = True
                new_insts.append(inst)
            if changed:
                bb.instructions = new_insts


def _pos_embed_np():
    """RelativeSinusoidalPositionalEmbedding table slice, [2S, HD] fp32."""
    num = 1201
    half = HD // 2
    freq = np.exp(np.arange(half, dtype=np.float32) * (-np.log(10000.0) / (half - 1)))
    pos = np.arange(-((num + 1) // 2), num // 2, dtype=np.float32)
    emb = pos[:, None] * freq[None, :]
    table = np.concatenate([np.sin(emb), np.cos(emb)], axis=1).astype(np.float32)
    table[0] = 0.0
    origin_shift = num // 2 + 1
    idx = np.arange(-S, S) + origin_shift
    return table[idx]  # [1024, 64]


# Band window start (pos-table columns) per q-tile:
#   Xt[p, j] = X[128t + p, e_t + j],  j in [0, 640)
#   BD[p, k] = Xt[p, CSKEW + k - p]
_E = [384 - 128 * t for t in range(QT)]


def _emit_body(nc, tc, tensors):
    (xT_d, wq_d, wv_d, posT2_d, rrb_d, rwb_d, identh_d, ident2_d,
     xskew_d, out_d) = tensors

    with tc.tile_pool(name="singles", bufs=1) as singles, \
         tc.tile_pool(name="sb_x", bufs=4) as sb_x, \
         tc.tile_pool(name="sb_bd", bufs=5) as sb_bd, \
         tc.tile_pool(name="sb_pt", bufs=3) as sb_pt, \
         tc.tile_pool(name="sb_small", bufs=2) as sb_small:

        # ---- persistent SBUF loads ---------------------------------------
        # Load order is tuned for ramp latency: the engine queues are
        # in-order, so only what q0/q1 need is issued before the first
        # projection; the bulk loads trail behind the first bias ops.
        xt_sb = singles.tile([128, KT, S], fp16, name="xt_sb")
        wq_sb = singles.tile([128, KT, D], fp16, name="wq_sb")
        wv_sb = singles.tile([128, KT, D], fp16, name="wv_sb")
        xt_r = xT_d.ap().rearrange("(kt p) s -> p kt s", p=128)
        wq_r = wq_d.ap().rearrange("(kt p) d -> p kt d", p=128)
        wv_r = wv_d.ap().rearrange("(kt p) d -> p kt d", p=128)
        for kt in range(KT):
            nc.sync.dma_start(out=xt_sb[:, kt], in_=xt_r[:, kt])
        nc.scalar.dma_start(out=wq_sb[:, 0], in_=wq_r[:, 0])
        nc.scalar.dma_start(out=wq_sb[:, 1], in_=wq_r[:, 1])
        rrb_sb = singles.tile([128, KT], f32, name="rrb_sb")
        nc.sync.dma_start(out=rrb_sb, in_=rrb_d.ap())
        rwb_sb = singles.tile([128, KT], f32, name="rwb_sb")
        nc.sync.dma_start(out=rwb_sb, in_=rwb_d.ap())
        posT2_sb = singles.tile([128, POSW], fp16, name="posT2_sb")
        identh_sb = singles.tile([128, 128], fp16, name="identh_sb")
        rwq_sb = singles.tile([128, KT, S], fp16, name="rwq_sb")
        rwq2_sb = singles.tile([128, KT, S], fp16, name="rwq2_sb")
        # v with a ones-column per head: [k-part, kc, head, HD+1]
        v2_sb = singles.tile([128, QT, H, HD + 1], bf16, name="v2_sb")
        out_sb = singles.tile([128, QT, D], fp16, name="out_sb")

        nc.gpsimd.memset(v2_sb[:, :, :, HD], 1.0)

        def emit_late_loads():
            for kt in range(2, KT):
                nc.scalar.dma_start(out=wq_sb[:, kt], in_=wq_r[:, kt])
            nc.scalar.dma_start(out=posT2_sb, in_=posT2_d.ap())
            nc.scalar.dma_start(out=identh_sb, in_=identh_d.ap())
            for kt in range(KT):
                nc.sync.dma_start(out=wv_sb[:, kt], in_=wv_r[:, kt])

        # skew reads are SBUF+DMA only (no PSUM) — the first two pairs'
        # reads are issued inside the phase-A loop right after their own
        # writes (the sync queue is in-order: emitted later, they would
        # head-of-line block behind all remaining writes' dependencies).
        def read_bd(h):
            t_ = sb_bd.tile(
                [128, QT, 512], fp16, name=f"bd_hh{h % 2}", tag=f"bd{h % 2}"
            )
            nc.sync.dma_start(
                out=t_,
                in_=bass.AP(
                    xskew_d,
                    h * QT * 128 * BAND + CSKEW,
                    [[BAND - 1, 128], [128 * BAND, QT], [1, 512]],
                ),
            )
            return t_

        bd_pre = {}

        # ================= PHASE A: projections + X bands =================
        # Dense N=512 matmul stream (keeps HAM warm); skew-band writes are
        # batched per head. PSUM: pXa 3 + pXb 2 + pQ 3 = 8 banks.
        with tc.tile_pool(name="pX", bufs=4, space="PSUM") as pX:
            pQ = pX

            def emit_q_group(dt):
                q_ps = pQ.tile([128, S], f32, name="q_ps", tag="px")
                for kt in range(KT):
                    nc.tensor.matmul(
                        q_ps,
                        lhsT=wq_sb[:, kt, dt * 128 : (dt + 1) * 128],
                        rhs=xt_sb[:, kt, :],
                        start=(kt == 0),
                        stop=(kt == KT - 1),
                    )
                nc.scalar.activation(
                    out=rwq_sb[:, dt, :], in_=q_ps,
                    func=mybir.ActivationFunctionType.Identity,
                    bias=rrb_sb[:, dt : dt + 1],
                )
                nc.vector.tensor_scalar_add(
                    out=rwq2_sb[:, dt, :], in0=q_ps,
                    scalar1=rwb_sb[:, dt : dt + 1],
                )

            def emit_v_group(vt, half):
                v_ps = pQ.tile([128, S], f32, name="v_ps", tag="px")
                for kt in range(KT):
                    nc.tensor.matmul(
                        v_ps,
                        lhsT=xt_sb[:, kt, vt * 128 : (vt + 1) * 128],
                        rhs=wv_sb[:, kt, half * 512 : (half + 1) * 512],
                        start=(kt == 0),
                        stop=(kt == KT - 1),
                    )
                nc.vector.tensor_copy(
                    out=v2_sb[:, vt, half * 8 : (half + 1) * 8, :HD],
                    in_=v_ps.rearrange("p (h d) -> p h d", d=HD),
                )

            emit_late_loads()
            emit_q_group(0)
            emit_q_group(1)
            for j in range(H // 2):
                pair = (2 * j, 2 * j + 1)
                if j + 2 < KT:
                    emit_q_group(j + 2)
                emit_v_group(j % QT, j // QT)
                x_hh = {}
                for h in pair:
                    x_hh[h] = sb_x.tile(
                        [128, QT, BAND], fp16, name=f"x_hh{h % 2}",
                        tag=f"x{h % 2}",
                    )
                for t in range(QT):
                    for h in pair:
                        qs = 64 * (h % 2)
                        dt = h // 2
                        e_t = _E[t]
                        lq2 = rwq2_sb[qs : qs + 64, dt, t * 128 : (t + 1) * 128]
                        x_ps = pX.tile([128, BAND], f32, name="x_ps", tag="px")
                        nc.tensor.matmul(
                            x_ps[:, :512], lhsT=lq2,
                            rhs=posT2_sb[qs : qs + 64, e_t : e_t + 512],
                            start=True, stop=True,
                        )
                        nc.tensor.matmul(
                            x_ps[:, 512:], lhsT=lq2,
                            rhs=posT2_sb[qs : qs + 64, e_t + 512 : e_t + 640],
                            start=True, stop=True,
                        )
                        if (t + h) % 2 == 0:
                            nc.scalar.copy(out=x_hh[h][:, t, :], in_=x_ps)
                        else:
                            nc.vector.tensor_copy(out=x_hh[h][:, t, :], in_=x_ps)
                for h in pair:
                    nc.sync.dma_start(
                        out=xskew_d.ap()[h].rearrange("t p j -> p t j"),
                        in_=x_hh[h],
                    )
                if j < 2:
                    for h in pair:
                        bd_pre[h] = read_bd(h)



        # ================= PHASE B: scores + softmax + AV =================
        # All-compute phase: skew reads stream in two pairs ahead, PE runs
        # AC^T + BD^T-accumulate + AV continuously.
        # PSUM: pST 3x2 + pAV 2 = 8 banks.
        out_r = out_d.ap().rearrange("(t p) d -> p t d", p=128)
        with tc.tile_pool(name="pST", bufs=3, space="PSUM") as pST, \
             tc.tile_pool(name="pAV", bufs=2, space="PSUM") as pAV:
            for j in range(H // 2):
                pair = (2 * j, 2 * j + 1)
                bd_hh = {h: bd_pre.pop(h) for h in pair}
                if 2 * j + 4 < H:
                    for h in (2 * j + 4, 2 * j + 5):
                        bd_pre[h] = read_bd(h)
                pt_hh = {}
                for h in pair:
                    pt_hh[h] = sb_pt.tile(
                        [128, QT, S], bf16, name=f"pt_hh{h % 2}",
                        tag=f"pt{h % 2}",
                    )
                for chunk in range(2):
                    st_ps = {}
                    for h in pair:
                        st_ps[h] = pST.tile([128, 1024], f32, name="st_ps", tag="pst")
                    # AC^T pair-packed: adjacent matmuls on partition halves
                    # 0-63 / 64-127 run concurrently in separate row groups.
                    for i, kc in enumerate((2 * chunk, 2 * chunk + 1)):
                        for h in pair:
                            qs = 64 * (h % 2)
                            dt = h // 2
                            nc.tensor.matmul(
                                st_ps[h][:, i * 512 : (i + 1) * 512],
                                lhsT=xt_sb[qs : qs + 64, dt, kc * 128 : (kc + 1) * 128],
                                rhs=rwq_sb[qs : qs + 64, dt, :],
                                start=True, stop=False,
                            )
                    for h in pair:
                        for i, kc in enumerate((2 * chunk, 2 * chunk + 1)):
                            for t in range(QT):
                                nc.tensor.matmul(
                                    st_ps[h][:, i * 512 + t * 128 : i * 512 + (t + 1) * 128],
                                    lhsT=bd_hh[h][:, t, kc * 128 : (kc + 1) * 128],
                                    rhs=identh_sb,
                                    start=False, stop=(t == QT - 1),
                                )
                    for h in pair:
                        nc.scalar.activation(
                            out=pt_hh[h][:, 2 * chunk : 2 * chunk + 2, :],
                            in_=st_ps[h],
                            func=mybir.ActivationFunctionType.Exp,
                        )

                for h in pair:
                    av_ps = pAV.tile([128, QT, HD + 2], f32, name="av_ps", tag="pav")
                    for t in range(QT):
                        for kc in range(QT):
                            nc.tensor.matmul(
                                av_ps[:, t, : HD + 1],
                                lhsT=pt_hh[h][:, kc, t * 128 : (t + 1) * 128],
                                rhs=v2_sb[:, kc, h, :],
                                start=(kc == 0), stop=(kc == QT - 1),
                            )
                    recip_sb = sb_small.tile(
                        [128, QT], f32, name="recip_sb", tag=f"recip{h % 2}"
                    )
                    nc.vector.reciprocal(out=recip_sb, in_=av_ps[:, :, HD])
                    nc.vector.tensor_tensor(
                        out=out_sb[:, :, h * HD : (h + 1) * HD],
                        in0=av_ps[:, :, :HD],
                        in1=recip_sb[:, :, None].to_broadcast((128, QT, HD)),
                        op=mybir.AluOpType.mult,
                    )

                for h in pair:
                    c0 = h * HD
                    nc.sync.dma_start(
                        out=out_r[:, :, c0 : c0 + HD],
                        in_=out_sb[:, :, c0 : c0 + HD],
                    )


def build_nc(n_repeat=1):
    nc = bass.Bass(
        trn_type="TRN2", target_bir_lowering=False, debug=False,
        num_devices=8, name="relattn",
    )
    xT_d = nc.dram_tensor("xt", [D, S], fp16, kind="ExternalInput")
    wq_d = nc.dram_tensor("wq", [D, D], fp16, kind="ExternalInput")
    wv_d = nc.dram_tensor("wv", [D, D], fp16, kind="ExternalInput")
    posT2_d = nc.dram_tensor("post2", [128, POSW], fp16, kind="ExternalInput")
    rrb_d = nc.dram_tensor("rrb", [128, KT], f32, kind="ExternalInput")
    rwb_d = nc.dram_tensor("rwb", [128, KT], f32, kind="ExternalInput")
    identh_d = nc.dram_tensor("identh", [128, 128], fp16, kind="ExternalInput")
    ident2_d = None
    xskew_d = nc.dram_tensor("xskew", [H, QT, 128, BAND], fp16)
    out_d = nc.dram_tensor("out", [S, D], fp16, kind="ExternalOutput")
    tensors = (xT_d, wq_d, wv_d, posT2_d, rrb_d, rwb_d, identh_d, ident2_d,
               xskew_d, out_d)

    with tile.TileContext(nc) as tc:
        _emit_body(nc, tc, tensors)
    _split_excess_waits(nc)
    return nc


def make_in_maps(inputs):
    x = np.asarray(inputs["x"], dtype=np.float32)
    Wqv = np.asarray(inputs["Wqv"], dtype=np.float32)
    rrb = np.asarray(inputs["r_r_bias"], dtype=np.float32)
    rwb = np.asarray(inputs["r_w_bias"], dtype=np.float32)

    pos = _pos_embed_np()                       # [1024, 64]
    posT = np.ascontiguousarray(pos.T)          # [64, 1024]
    posT2 = np.concatenate([posT, posT], axis=0).astype(np.float16)
    wq = np.ascontiguousarray(Wqv[:, :D]).astype(np.float16)
    wv = np.ascontiguousarray(Wqv[:, D:]).astype(np.float16)
    rrb_col = np.ascontiguousarray(rrb.reshape(KT, 128).T)
    rwb_col = np.ascontiguousarray(rwb.reshape(KT, 128).T)

    in_maps = []
    for b in range(B):
        in_maps.append({
            "xt": np.ascontiguousarray(x[b].T).astype(np.float16),
            "wq": wq,
            "wv": wv,
            "post2": posT2,
            "rrb": rrb_col,
            "rwb": rwb_col,
            "identh": np.eye(128, dtype=np.float16),
        })
    return in_maps


_cached = {}


def run(inputs, n_repeat=1):
    if n_repeat not in _cached:
        _cached[n_repeat] = build_nc(n_repeat)
    nc = _cached[n_repeat]
    in_maps = make_in_maps(inputs)
    res = run_bass_kernel_spmd(nc, in_maps, core_ids=list(range(B)))
    out = np.stack([res.results[b]["out"] for b in range(B)], axis=0)
    return out.astype(np.float32)


def kernel(**inputs) -> np.ndarray:
    return run(inputs, n_repeat=1)
